# revision 38
# baseline (speedup 1.0000x reference)
"""Causal multi-head attention (B=2, S=2048, D=1024, H=16, DH=64) on 8 TRN2 cores.

Sharding: core c handles batch b = c//4 and head group g = c%4 (4 heads, 256
feature cols).  Each core computes Q/K/V projections for its heads, causal
attention, and a partial output projection; the host sums the 4 partials per
batch.

v2 layout (per core), all SBUF operands bf16 (PSUM accum stays f32):
  - host pre-transposes X[b] -> XT [D, S] and weight slices, casting to bf16.
  - Q^T, K^T kept as [o, s] (2 heads per 128-partition tile), V as [s, o]
    with a ones column per head (65-row PV stationary emits the softmax
    denominator as acc row 64 for free).
  - logits computed transposed (L^T = K_h Q_h^T) into PSUM f32; exp on ACT
    writes bf16 pt tiles; causal triangle masking via gpsimd affine_select
    in place (only the [128,128] diagonal sub-blocks); PV accumulates in
    PSUM f32.
  - causal fine-grain: for diagonal k-chunks only q-cols >= 128j-512t are
    computed in logits/exp/PV (bf16 matmuls run 1 cycle/col at any width).
  - normalization: DVE reciprocal of the denominator row, PE outer-product
    broadcast (ones[1,64]^T @ recip[1,512]), DVE multiply into outTn bf16;
    head B of each pair partition-shifted 0-63 -> 64-127 via SBUF-SBUF DMA.
  - output projection interleaved into the attention t-loop (4 s-chunks per
    tile), PSUM f32 -> ys staging -> HBM f32.
"""

import os
import numpy as np

B, S, D = 2, 2048, 1024
H, DH = 16, 64
NCORES = 8
GROUPS = 4          # head groups (one per core within a batch)
HPC = H // GROUPS   # heads per core = 4
O = HPC * DH        # per-core feature cols = 256
DC = D // 128       # contraction chunks = 8
NQT = S // 512      # q tiles = 4
NST = S // 128      # s chunks = 16

_PROGRAM = None
LAST_RESULTS = None  # stashed BassKernelResults for test harness introspection


def _build_program(loop_n=1):
    import concourse.bass as bass
    import concourse.tile as tile
    from concourse import bacc, mybir
    from contextlib import ExitStack

    f32 = mybir.dt.float32
    bf16 = mybir.dt.bfloat16
    ts = bass.ts
    Exp = mybir.ActivationFunctionType.Exp
    Copy = mybir.ActivationFunctionType.Copy

    nc = bacc.Bacc("TRN2", target_bir_lowering=False, debug=False,
                   num_devices=NCORES)

    xt = nc.dram_tensor("xt", [D, S], bf16, kind="ExternalInput").ap()
    wqt = nc.dram_tensor("wqt", [D, O], bf16, kind="ExternalInput").ap()
    wkt = nc.dram_tensor("wkt", [D, O], bf16, kind="ExternalInput").ap()
    wvt = nc.dram_tensor("wvt", [D, O], bf16, kind="ExternalInput").ap()
    wot = nc.dram_tensor("wot", [O, D], bf16, kind="ExternalInput").ap()
    y = nc.dram_tensor("y", [S, D], f32, kind="ExternalOutput").ap()

    with tile.TileContext(nc) as tc, ExitStack() as ctx:
        per = ctx.enter_context(tc.tile_pool(name="per", bufs=1))
        xtp = ctx.enter_context(tc.tile_pool(name="xtp", bufs=2))
        work = ctx.enter_context(tc.tile_pool(name="work", bufs=6))
        ps_sm = ctx.enter_context(tc.tile_pool(name="ps_sm", bufs=4, space="PSUM"))
        ps_lt = ctx.enter_context(tc.tile_pool(name="ps_lt", bufs=2, space="PSUM"))

        # ---- persistent tiles -------------------------------------------------
        wq_sb = per.tile([128, DC, O], bf16, tag="wq")
        wk_sb = per.tile([128, DC, O], bf16, tag="wk")
        wv_sb = per.tile([128, DC, O], bf16, tag="wv")
        wo_sb = per.tile([128, 2, D], bf16, tag="wo")
        qT = per.tile([128, 2, S], bf16, tag="qT")
        kT = per.tile([128, 2, S], bf16, tag="kT")
        # V with a ones column per head: [s-chunk, head, 64 V + 1 one]
        vones = per.tile([128, NST, HPC, DH + 1], bf16, tag="vones")
        outTn = per.tile([128, 2, S], bf16, tag="outTn")

        SH = S // 2

        # ones row on partition 64, cols 0..63 (bf16) for the outer-product
        # broadcast of softmax reciprocals: bcast = ones[1,64].T @ recip[1,512]
        ones_bc = per.tile([128, DH], bf16, tag="ones_bc")
        nc.vector.memset(ones_bc[DH:DH + 1, :], 1.0)
        # [64,64] identity (bf16) for PE partition-shift of head-B outputs
        ident = per.tile([128, DH], bf16, tag="ident")
        nc.vector.memset(ident[0:DH, :], 1.0)
        nc.gpsimd.affine_select(
            out=ident[0:DH, :], in_=ident[0:DH, :],
            compare_op=mybir.AluOpType.is_equal, fill=0.0,
            base=0, pattern=[[1, DH]], channel_multiplier=-1)
        for st in range(NST):
            nc.vector.memset(vones[:, st, :, DH:DH + 1], 1.0)

        # ---- phase 0 helper: input DMAs, chunked so the first projection
        # matmuls only wait on their own contraction chunk ---------------------
        def load_inputs():
            wkr = wkt.rearrange("(c p) o -> p c o", p=128)
            wqr = wqt.rearrange("(c p) o -> p c o", p=128)
            wvr = wvt.rearrange("(c p) o -> p c o", p=128)
            xt_sbs = [xtp.tile([128, DC, SH], bf16, tag="xt", name="xt")
                      for _ in range(2)]
            for dc in range(DC):
                nc.sync.dma_start(wk_sb[:, dc, :], wkr[:, dc, :])
                nc.sync.dma_start(
                    xt_sbs[0][:, dc, :], xt[dc * 128:(dc + 1) * 128, 0:SH])
            for dc in range(DC):
                nc.sync.dma_start(wq_sb[:, dc, :], wqr[:, dc, :])
                nc.sync.dma_start(wv_sb[:, dc, :], wvr[:, dc, :])
            for dc in range(DC):
                nc.sync.dma_start(
                    xt_sbs[1][:, dc, :], xt[dc * 128:(dc + 1) * 128, SH:S])
            nc.sync.dma_start(wo_sb[:], wot.rearrange("(c p) m -> p c m", p=128))
            return xt_sbs

        # ---- phase 1 helper: K^T, Q^T, V projections for one s-half ----------
        def proj_half(xt_sbs, sh, qts=(0, 1)):
            xt_sb = xt_sbs[sh]
            for w_sb, dst in ((wk_sb, kT), (wq_sb, qT)):
                for pt_i in range(2):
                    for qt in qts:
                        ps = ps_sm.tile([128, 512], f32, tag="sm", name="mm")
                        for dc in range(DC):
                            nc.tensor.matmul(
                                ps[:],
                                w_sb[:, dc, ts(pt_i, 128)],
                                xt_sb[:, dc, ts(qt, 512)],
                                start=(dc == 0), stop=(dc == DC - 1),
                            )
                        nc.vector.tensor_copy(
                            dst[:, pt_i, ts(sh * 2 + qt, 512)], ps[:])
            for st_l in range(4 * qts[0], 4 * qts[-1] + 4):
                st = sh * (SH // 128) + st_l
                ps = ps_sm.tile([128, O], f32, tag="sm", name="mm")
                for dc in range(DC):
                    nc.tensor.matmul(
                        ps[:],
                        xt_sb[:, dc, ts(st_l, 128)],
                        wv_sb[:, dc, :],
                        start=(dc == 0), stop=(dc == DC - 1),
                    )
                nc.vector.tensor_copy(
                    vones[:, st, :, 0:DH],
                    ps[:].rearrange("p (h d) -> p h d", h=HPC),
                )

        # ---- phase 3 helper: output projection for one q tile's 4 s-chunks ---
        def out_proj(t):
            for st in range(4 * t, 4 * t + 4):
                ys = work.tile([128, 1024], f32, tag="ystage")
                for mt in range(2):
                    ps = ps_sm.tile([128, 512], f32, tag="sm", name="mm")
                    for pair in range(2):
                        nc.tensor.matmul(
                            ps[:],
                            outTn[:, pair, ts(st, 128)],
                            wo_sb[:, pair, ts(mt, 512)],
                            start=(pair == 0), stop=(pair == 1),
                        )
                    nc.vector.tensor_copy(ys[:, ts(mt, 512)], ps[:])
                nc.sync.dma_start(y[ts(st, 128), :], ys[:])

        # ---- phase 2 helper: attention for one q tile; `mid` (the previous
        # tile's output projection) is emitted between the two head pairs so
        # its matmuls fill PE gaps while this tile's exps run -----------------
        def attn_tile(t, mid=None, carry_in=None):
            carry = carry_in
            for pair in range(2):
                if pair == 1 and mid is not None:
                    mid()
                accs = [ps_sm.tile([DH + 1, 512], f32, tag="sm", name="acc")
                        for _ in range(2)]
                njp = 2 * t + 2  # j-pairs covering k chunks 0..4t+3
                for jg in range(njp):
                    if jg == 1 and carry is not None:
                        carry()  # deferred previous-pair normalization
                        carry = None
                    j0, j1 = 2 * jg, 2 * jg + 1
                    # fine-grain causal: for diagonal k-chunks only q-cols
                    # >= 128j - 512t participate
                    c0 = max(0, 128 * j0 - 512 * t)
                    c1 = max(0, 128 * j1 - 512 * t)
                    lts = [ps_lt.tile([128, 1024], f32, tag="lt", name="lt")
                           for _ in range(2)]
                    for j_half, j, c in ((0, j0, c0), (1, j1, c1)):
                        for h01 in range(2):
                            bp = 64 * h01
                            nc.tensor.matmul(
                                lts[h01][:, j_half * 512 + c:(j_half + 1) * 512],
                                kT[bp:bp + 64, pair, ts(j, 128)],
                                qT[bp:bp + 64, pair, 512 * t + c:512 * (t + 1)],
                                start=True, stop=True,
                            )
                    for h01 in range(2):
                        h = 2 * pair + h01
                        pt = work.tile([128, 1024], bf16, tag="pt", bufs=6)
                        if c1 == 0:
                            nc.scalar.activation(pt[:], lts[h01][:], Exp,
                                                 scale=DH ** -0.5)
                        else:
                            nc.scalar.activation(
                                pt[:, c0:512], lts[h01][:, c0:512], Exp,
                                scale=DH ** -0.5)
                            nc.scalar.activation(
                                pt[:, 512 + c1:1024], lts[h01][:, 512 + c1:1024],
                                Exp, scale=DH ** -0.5)
                        # zero the strictly-upper triangle of diagonal blocks
                        for j_half, j, c in ((0, j0, c0), (1, j1, c1)):
                            if j >= 4 * t:
                                base = j_half * 512 + c
                                nc.gpsimd.affine_select(
                                    out=pt[:, base:base + 128],
                                    in_=pt[:, base:base + 128],
                                    compare_op=mybir.AluOpType.is_ge,
                                    fill=0.0,
                                    base=0,
                                    pattern=[[1, 128]],
                                    channel_multiplier=-1,
                                )
                        for j_half, j, c in ((0, j0, c0), (1, j1, c1)):
                            nc.tensor.matmul(
                                accs[h01][:, c:512],
                                vones[:, j, h, :],
                                pt[:, j_half * 512 + c:(j_half + 1) * 512],
                                start=(jg == 0 and j_half == 0),
                                stop=(jg == njp - 1 and j_half == 1),
                            )
                def norm(accs=accs, pair=pair):
                    for h01 in range(2):
                        acc = accs[h01]
                        recip = work.tile([128, 512], bf16, tag="recip")
                        with nc.allow_low_precision(
                                reason="softmax reciprocal feeds bf16 pipeline"):
                            nc.vector.reciprocal(
                                recip[DH:DH + 1, :], acc[DH:DH + 1, :])
                        # broadcast recip row to partitions 0..63 via PE outer
                        # product (ones[1,64].T @ recip[1,512])
                        bc_ps = ps_lt.tile([128, 1024], f32, tag="lt", name="bcps")
                        nc.tensor.matmul(bc_ps[0:DH, 0:512],
                                         ones_bc[DH:DH + 1, :],
                                         recip[DH:DH + 1, :],
                                         start=True, stop=True)
                        bcast = work.tile([128, 512], bf16, tag="bcast")
                        nc.vector.tensor_copy(bcast[0:DH, :], bc_ps[0:DH, 0:512])
                        if h01 == 0:
                            dst = outTn[0:DH, pair, ts(t, 512)]
                            nc.vector.tensor_mul(dst, acc[0:DH, :],
                                                 bcast[0:DH, :])
                        else:
                            sg = work.tile([128, 512], bf16, tag="stg")
                            nc.vector.tensor_mul(sg[0:DH, :], acc[0:DH, :],
                                                 bcast[0:DH, :])
                            # partition shift 0-63 -> 64-127 via SBUF->SBUF DMA
                            nc.sync.dma_start(
                                outTn[DH:128, pair, ts(t, 512)], sg[0:DH, :])
                if pair == 0:
                    norm()
                else:
                    carry = norm
            return carry

        # ---- emission: interleave second projection half between q tiles,
        # and each tile's output projection into the next tile ----------------
        def body():
            xt_sbs = load_inputs()
            proj_half(xt_sbs, 0)
            c = attn_tile(0)
            c = attn_tile(1, mid=lambda: out_proj(0), carry_in=c)
            proj_half(xt_sbs, 1, qts=(0,))
            c = attn_tile(2, mid=lambda: out_proj(1), carry_in=c)
            proj_half(xt_sbs, 1, qts=(1,))
            c = attn_tile(3, mid=lambda: out_proj(2), carry_in=c)
            c()
            out_proj(3)

        if loop_n == 1:
            body()
        else:
            with tc.For_i(0, loop_n, 1):
                body()

    nc.compile()
    return nc


def _get_program(loop_n=1):
    global _PROGRAM
    if _PROGRAM is None:
        _PROGRAM = {}
    if loop_n not in _PROGRAM:
        _PROGRAM[loop_n] = _build_program(loop_n)
    return _PROGRAM[loop_n]


def kernel(X, Wq, Wk, Wv, Wo):
    global LAST_RESULTS
    from concourse.bass_utils import run_bass_kernel_spmd

    X = np.asarray(X, dtype=np.float32)
    Wq = np.asarray(Wq, dtype=np.float32)
    Wk = np.asarray(Wk, dtype=np.float32)
    Wv = np.asarray(Wv, dtype=np.float32)
    Wo = np.asarray(Wo, dtype=np.float32)

    nc = _get_program()
    in_maps = _make_in_maps(X, Wq, Wk, Wv, Wo)
    res = run_bass_kernel_spmd(nc, in_maps, list(range(NCORES)))
    LAST_RESULTS = res

    out = np.empty((B, S, D), dtype=np.float32)
    for b in range(B):
        acc = res.results[b * GROUPS]["y"].astype(np.float32)
        for g in range(1, GROUPS):
            acc = acc + res.results[b * GROUPS + g]["y"]
        out[b] = acc
    return out


def _make_in_maps(X, Wq, Wk, Wv, Wo):
    import ml_dtypes
    bf16 = ml_dtypes.bfloat16
    xts = [np.ascontiguousarray(X[b].T).astype(bf16) for b in range(B)]
    in_maps = []
    for c in range(NCORES):
        b, g = divmod(c, GROUPS)
        rows = slice(g * O, (g + 1) * O)
        in_maps.append({
            "xt": xts[b],
            "wqt": np.ascontiguousarray(Wq[rows, :].T).astype(bf16),
            "wkt": np.ascontiguousarray(Wk[rows, :].T).astype(bf16),
            "wvt": np.ascontiguousarray(Wv[rows, :].T).astype(bf16),
            "wot": np.ascontiguousarray(Wo[:, rows].T).astype(bf16),
        })
    return in_maps


def build_timed_callable(in_maps=None, loop_n=1):
    """Build the same sharded jit callable bass2jax uses, with inputs
    pre-placed on the 8 devices, for repeat-timing the NEFF execution.

    With loop_n=K the device program wraps the whole kernel body (including
    input DMAs) in a K-iteration hardware loop, so per-exec device time can
    be measured as a slope between two loop counts, cancelling the (large,
    noisy) axon dispatch overhead."""
    import jax
    import numpy as np
    from jax.sharding import Mesh, PartitionSpec, NamedSharding
    from jax.experimental.shard_map import shard_map
    from concourse import bass2jax, mybir

    nc = _get_program(loop_n)
    bass2jax.install_neuronx_cc_hook()

    if in_maps is None:
        import test as _t
        inputs, _ = _t.get_reference_data()
        in_maps = _make_in_maps(**inputs)

    partition_name = (
        nc.partition_id_tensor.name if nc.partition_id_tensor else None)
    in_names, out_names, out_avals, zero_shapes = [], [], [], []
    for alloc in nc.m.functions[0].allocations:
        if not isinstance(alloc, mybir.MemoryLocationSet):
            continue
        name = alloc.memorylocations[0].name
        if alloc.kind == "ExternalInput":
            if name != partition_name:
                in_names.append(name)
        elif alloc.kind == "ExternalOutput":
            out_names.append(name)
            shape = tuple(alloc.tensor_shape)
            out_avals.append(
                jax.core.ShapedArray(shape, mybir.dt.np(alloc.dtype)))
            zero_shapes.append((NCORES * shape[0], *shape[1:]))
    n_params = len(in_names)
    n_out = len(out_names)
    # operand order: inputs, donated zero outputs, partition ids (last, so
    # the hook's operand_ids[:-1] parameter-order check sees params 0..N-1)
    all_names = in_names + out_names
    if partition_name is not None:
        all_names = all_names + [partition_name]
    donate = tuple(range(n_params, n_params + n_out))

    def _body(*args):
        outs = bass2jax._bass_exec_p.bind(
            *args,
            out_avals=tuple(out_avals),
            in_names=tuple(all_names),
            out_names=tuple(out_names),
            lowering_input_output_aliases=(),
            sim_require_finite=True,
            sim_require_nnan=True,
            nc=nc,
        )
        return tuple(outs)

    devices = jax.devices()[:NCORES]
    mesh = Mesh(np.asarray(devices), ("core",))
    spec = PartitionSpec("core")
    n_extra = 1 if partition_name is not None else 0
    fn = jax.jit(
        shard_map(_body, mesh=mesh,
                  in_specs=(spec,) * (n_params + n_out + n_extra),
                  out_specs=(spec,) * n_out, check_rep=False),
        donate_argnums=donate, keep_unused=True,
    )
    sharding = NamedSharding(mesh, spec)
    concat_in = [
        jax.device_put(
            np.concatenate([np.asarray(in_maps[c][nm]) for c in range(NCORES)],
                           axis=0), sharding)
        for nm in in_names
    ]
    if partition_name is not None:
        pid = jax.device_put(
            np.arange(NCORES, dtype=np.uint32).reshape(NCORES, 1), sharding)
        fn_inner = fn
        fn = lambda *args: fn_inner(*args, pid)
    return fn, concat_in, [(s, sharding) for s in zero_shapes]


# revision 39
# speedup vs baseline: 1.0865x; 1.0865x over previous
"""Causal multi-head attention (B=2, S=2048, D=1024, H=16, DH=64) on 8 TRN2 cores.

Sharding: core c handles batch b = c//4 and head group g = c%4 (4 heads, 256
feature cols).  Each core computes Q/K/V projections for its heads, causal
attention, and a partial output projection; the host sums the 4 partials per
batch.

v2 layout (per core), all SBUF operands bf16 (PSUM accum stays f32):
  - host pre-transposes X[b] -> XT [D, S] and weight slices, casting to bf16.
  - Q^T, K^T kept as [o, s] (2 heads per 128-partition tile), V as [s, o]
    with a ones column per head (65-row PV stationary emits the softmax
    denominator as acc row 64 for free).
  - logits computed transposed (L^T = K_h Q_h^T) into PSUM f32; exp on ACT
    writes bf16 pt tiles; causal triangle masking via gpsimd affine_select
    in place (only the [128,128] diagonal sub-blocks); PV accumulates in
    PSUM f32.
  - causal fine-grain: for diagonal k-chunks only q-cols >= 128j-512t are
    computed in logits/exp/PV (bf16 matmuls run 1 cycle/col at any width).
  - normalization: DVE reciprocal of the denominator row, PE outer-product
    broadcast (ones[1,64]^T @ recip[1,512]), DVE multiply into outTn bf16;
    head B of each pair partition-shifted 0-63 -> 64-127 via SBUF-SBUF DMA.
  - output projection interleaved into the attention t-loop (4 s-chunks per
    tile), PSUM f32 -> ys staging -> HBM f32.
"""

import os
import numpy as np

B, S, D = 2, 2048, 1024
H, DH = 16, 64
NCORES = 8
GROUPS = 4          # head groups (one per core within a batch)
HPC = H // GROUPS   # heads per core = 4
O = HPC * DH        # per-core feature cols = 256
DC = D // 128       # contraction chunks = 8
NQT = S // 512      # q tiles = 4
NST = S // 128      # s chunks = 16

_PROGRAM = None
LAST_RESULTS = None  # stashed BassKernelResults for test harness introspection


def _build_program(loop_n=1):
    import concourse.bass as bass
    import concourse.tile as tile
    from concourse import bacc, mybir
    from contextlib import ExitStack

    f32 = mybir.dt.float32
    bf16 = mybir.dt.bfloat16
    ts = bass.ts
    Exp = mybir.ActivationFunctionType.Exp
    Copy = mybir.ActivationFunctionType.Copy

    nc = bacc.Bacc("TRN2", target_bir_lowering=False, debug=False,
                   num_devices=NCORES)

    xt = nc.dram_tensor("xt", [D, S], bf16, kind="ExternalInput").ap()
    wqt = nc.dram_tensor("wqt", [D, O], bf16, kind="ExternalInput").ap()
    wkt = nc.dram_tensor("wkt", [D, O], bf16, kind="ExternalInput").ap()
    wvt = nc.dram_tensor("wvt", [D, O], bf16, kind="ExternalInput").ap()
    wot = nc.dram_tensor("wot", [O, D], bf16, kind="ExternalInput").ap()
    y = nc.dram_tensor("y", [S, D], f32, kind="ExternalOutput").ap()

    with tile.TileContext(nc) as tc, ExitStack() as ctx:
        per = ctx.enter_context(tc.tile_pool(name="per", bufs=1))
        xtp = ctx.enter_context(tc.tile_pool(name="xtp", bufs=2))
        work = ctx.enter_context(tc.tile_pool(name="work", bufs=6))
        ps_sm = ctx.enter_context(tc.tile_pool(name="ps_sm", bufs=4, space="PSUM"))
        ps_lt = ctx.enter_context(tc.tile_pool(name="ps_lt", bufs=2, space="PSUM"))

        # ---- persistent tiles -------------------------------------------------
        wq_sb = per.tile([128, DC, O], bf16, tag="wq")
        wk_sb = per.tile([128, DC, O], bf16, tag="wk")
        wv_sb = per.tile([128, DC, O], bf16, tag="wv")
        wo_sb = per.tile([128, 2, D], bf16, tag="wo")
        qT = per.tile([128, 2, S], bf16, tag="qT")
        kT = per.tile([128, 2, S], bf16, tag="kT")
        # V with a ones column per head: [s-chunk, head, 64 V + 1 one]
        vones = per.tile([128, NST, HPC, DH + 1], bf16, tag="vones")
        outTn = per.tile([128, 2, S], bf16, tag="outTn")

        SH = S // 2

        # ones row on partition 64, cols 0..63 (bf16) for the outer-product
        # broadcast of softmax reciprocals: bcast = ones[1,64].T @ recip[1,512]
        ones_bc = per.tile([128, DH], bf16, tag="ones_bc")
        nc.vector.memset(ones_bc[DH:DH + 1, :], 1.0)
        # [64,64] identity (bf16) for PE partition-shift of head-B outputs
        ident = per.tile([128, DH], bf16, tag="ident")
        nc.vector.memset(ident[0:DH, :], 1.0)
        nc.gpsimd.affine_select(
            out=ident[0:DH, :], in_=ident[0:DH, :],
            compare_op=mybir.AluOpType.is_equal, fill=0.0,
            base=0, pattern=[[1, DH]], channel_multiplier=-1)
        for st in range(NST):
            nc.vector.memset(vones[:, st, :, DH:DH + 1], 1.0)

        # ---- phase 0 helper: input DMAs, chunked so the first projection
        # matmuls only wait on their own contraction chunk ---------------------
        def load_inputs():
            wkr = wkt.rearrange("(c p) o -> p c o", p=128)
            wqr = wqt.rearrange("(c p) o -> p c o", p=128)
            wvr = wvt.rearrange("(c p) o -> p c o", p=128)
            xt_sbs = [xtp.tile([128, DC, SH], bf16, tag="xt", name="xt")
                      for _ in range(2)]
            for dc in range(DC):
                nc.sync.dma_start(wk_sb[:, dc, :], wkr[:, dc, :])
                nc.sync.dma_start(
                    xt_sbs[0][:, dc, :], xt[dc * 128:(dc + 1) * 128, 0:SH])
            for dc in range(DC):
                nc.sync.dma_start(wq_sb[:, dc, :], wqr[:, dc, :])
                nc.sync.dma_start(wv_sb[:, dc, :], wvr[:, dc, :])
            for dc in range(DC):
                nc.sync.dma_start(
                    xt_sbs[1][:, dc, :], xt[dc * 128:(dc + 1) * 128, SH:S])
            nc.sync.dma_start(wo_sb[:], wot.rearrange("(c p) m -> p c m", p=128))
            return xt_sbs

        # ---- phase 1 helper: K^T, Q^T, V projections for one s-half ----------
        def proj_half(xt_sbs, sh, qts=(0, 1)):
            xt_sb = xt_sbs[sh]
            for w_sb, dst in ((wk_sb, kT), (wq_sb, qT)):
                for pt_i in range(2):
                    for qt in qts:
                        ps = ps_sm.tile([128, 512], f32, tag="sm", name="mm")
                        for dc in range(DC):
                            nc.tensor.matmul(
                                ps[:],
                                w_sb[:, dc, ts(pt_i, 128)],
                                xt_sb[:, dc, ts(qt, 512)],
                                start=(dc == 0), stop=(dc == DC - 1),
                            )
                        # psum f32 -> bf16 persistent; ACT so DVE stays free
                        nc.scalar.activation(
                            dst[:, pt_i, ts(sh * 2 + qt, 512)], ps[:], Copy)
            for st_l in range(4 * qts[0], 4 * qts[-1] + 4):
                st = sh * (SH // 128) + st_l
                ps = ps_sm.tile([128, O], f32, tag="sm", name="mm")
                for dc in range(DC):
                    nc.tensor.matmul(
                        ps[:],
                        xt_sb[:, dc, ts(st_l, 128)],
                        wv_sb[:, dc, :],
                        start=(dc == 0), stop=(dc == DC - 1),
                    )
                nc.vector.tensor_copy(
                    vones[:, st, :, 0:DH],
                    ps[:].rearrange("p (h d) -> p h d", h=HPC),
                )

        # ---- phase 3 helper: output projection for one q tile's 4 s-chunks ---
        def out_proj(t):
            for st in range(4 * t, 4 * t + 4):
                ys = work.tile([128, 1024], f32, tag="ystage")
                for mt in range(2):
                    ps = ps_sm.tile([128, 512], f32, tag="sm", name="mm")
                    for pair in range(2):
                        nc.tensor.matmul(
                            ps[:],
                            outTn[:, pair, ts(st, 128)],
                            wo_sb[:, pair, ts(mt, 512)],
                            start=(pair == 0), stop=(pair == 1),
                        )
                    nc.vector.tensor_copy(ys[:, ts(mt, 512)], ps[:])
                nc.sync.dma_start(y[ts(st, 128), :], ys[:])

        # ---- phase 2 helper: attention for one q tile; `mid` (the previous
        # tile's output projection) is emitted between the two head pairs so
        # its matmuls fill PE gaps while this tile's exps run -----------------
        def attn_tile(t, mid=None, carry_in=None):
            carry = carry_in
            for pair in range(2):
                if pair == 1 and mid is not None:
                    mid()
                accs = [ps_sm.tile([DH + 1, 512], f32, tag="sm", name="acc")
                        for _ in range(2)]
                njp = 2 * t + 2  # j-pairs covering k chunks 0..4t+3
                for jg in range(njp):
                    if jg == 1 and carry is not None:
                        carry()  # deferred previous-pair normalization
                        carry = None
                    j0, j1 = 2 * jg, 2 * jg + 1
                    # fine-grain causal: for diagonal k-chunks only q-cols
                    # >= 128j - 512t participate
                    c0 = max(0, 128 * j0 - 512 * t)
                    c1 = max(0, 128 * j1 - 512 * t)
                    lts = [ps_lt.tile([128, 1024], f32, tag="lt", name="lt")
                           for _ in range(2)]
                    for j_half, j, c in ((0, j0, c0), (1, j1, c1)):
                        for h01 in range(2):
                            bp = 64 * h01
                            nc.tensor.matmul(
                                lts[h01][:, j_half * 512 + c:(j_half + 1) * 512],
                                kT[bp:bp + 64, pair, ts(j, 128)],
                                qT[bp:bp + 64, pair, 512 * t + c:512 * (t + 1)],
                                start=True, stop=True,
                            )
                    for h01 in range(2):
                        h = 2 * pair + h01
                        pt = work.tile([128, 1024], bf16, tag="pt", bufs=6)
                        if c1 == 0:
                            nc.scalar.activation(pt[:], lts[h01][:], Exp,
                                                 scale=DH ** -0.5)
                        else:
                            nc.scalar.activation(
                                pt[:, c0:512], lts[h01][:, c0:512], Exp,
                                scale=DH ** -0.5)
                            nc.scalar.activation(
                                pt[:, 512 + c1:1024], lts[h01][:, 512 + c1:1024],
                                Exp, scale=DH ** -0.5)
                        # zero the strictly-upper triangle of diagonal blocks
                        for j_half, j, c in ((0, j0, c0), (1, j1, c1)):
                            if j >= 4 * t:
                                base = j_half * 512 + c
                                nc.gpsimd.affine_select(
                                    out=pt[:, base:base + 128],
                                    in_=pt[:, base:base + 128],
                                    compare_op=mybir.AluOpType.is_ge,
                                    fill=0.0,
                                    base=0,
                                    pattern=[[1, 128]],
                                    channel_multiplier=-1,
                                )
                        for j_half, j, c in ((0, j0, c0), (1, j1, c1)):
                            nc.tensor.matmul(
                                accs[h01][:, c:512],
                                vones[:, j, h, :],
                                pt[:, j_half * 512 + c:(j_half + 1) * 512],
                                start=(jg == 0 and j_half == 0),
                                stop=(jg == njp - 1 and j_half == 1),
                            )
                def norm(accs=accs, pair=pair):
                    for h01 in range(2):
                        acc = accs[h01]
                        recip = work.tile([128, 512], bf16, tag="recip")
                        with nc.allow_low_precision(
                                reason="softmax reciprocal feeds bf16 pipeline"):
                            nc.vector.reciprocal(
                                recip[DH:DH + 1, :], acc[DH:DH + 1, :])
                        # broadcast recip row to partitions 0..63 via PE outer
                        # product (ones[1,64].T @ recip[1,512])
                        bc_ps = ps_lt.tile([128, 1024], f32, tag="lt", name="bcps")
                        nc.tensor.matmul(bc_ps[0:DH, 0:512],
                                         ones_bc[DH:DH + 1, :],
                                         recip[DH:DH + 1, :],
                                         start=True, stop=True)
                        bcast = work.tile([128, 512], bf16, tag="bcast")
                        nc.vector.tensor_copy(bcast[0:DH, :], bc_ps[0:DH, 0:512])
                        if h01 == 0:
                            dst = outTn[0:DH, pair, ts(t, 512)]
                            nc.vector.tensor_mul(dst, acc[0:DH, :],
                                                 bcast[0:DH, :])
                        else:
                            sg = work.tile([128, 512], bf16, tag="stg")
                            nc.vector.tensor_mul(sg[0:DH, :], acc[0:DH, :],
                                                 bcast[0:DH, :])
                            # partition shift 0-63 -> 64-127 via SBUF->SBUF DMA
                            nc.sync.dma_start(
                                outTn[DH:128, pair, ts(t, 512)], sg[0:DH, :])
                if pair == 0:
                    norm()
                else:
                    carry = norm
            return carry

        # ---- emission: interleave second projection half between q tiles,
        # and each tile's output projection into the next tile ----------------
        def body():
            xt_sbs = load_inputs()
            proj_half(xt_sbs, 0)
            c = attn_tile(0)
            c = attn_tile(1, mid=lambda: out_proj(0), carry_in=c)
            proj_half(xt_sbs, 1, qts=(0,))
            c = attn_tile(2, mid=lambda: out_proj(1), carry_in=c)
            proj_half(xt_sbs, 1, qts=(1,))
            c = attn_tile(3, mid=lambda: out_proj(2), carry_in=c)
            c()
            out_proj(3)

        if loop_n == 1:
            body()
        else:
            with tc.For_i(0, loop_n, 1):
                body()

    nc.compile()
    return nc


def _get_program(loop_n=1):
    global _PROGRAM
    if _PROGRAM is None:
        _PROGRAM = {}
    if loop_n not in _PROGRAM:
        _PROGRAM[loop_n] = _build_program(loop_n)
    return _PROGRAM[loop_n]


def kernel(X, Wq, Wk, Wv, Wo):
    global LAST_RESULTS
    from concourse.bass_utils import run_bass_kernel_spmd

    X = np.asarray(X, dtype=np.float32)
    Wq = np.asarray(Wq, dtype=np.float32)
    Wk = np.asarray(Wk, dtype=np.float32)
    Wv = np.asarray(Wv, dtype=np.float32)
    Wo = np.asarray(Wo, dtype=np.float32)

    nc = _get_program()
    in_maps = _make_in_maps(X, Wq, Wk, Wv, Wo)
    res = run_bass_kernel_spmd(nc, in_maps, list(range(NCORES)))
    LAST_RESULTS = res

    out = np.empty((B, S, D), dtype=np.float32)
    for b in range(B):
        acc = res.results[b * GROUPS]["y"].astype(np.float32)
        for g in range(1, GROUPS):
            acc = acc + res.results[b * GROUPS + g]["y"]
        out[b] = acc
    return out


def _make_in_maps(X, Wq, Wk, Wv, Wo):
    import ml_dtypes
    bf16 = ml_dtypes.bfloat16
    xts = [np.ascontiguousarray(X[b].T).astype(bf16) for b in range(B)]
    in_maps = []
    for c in range(NCORES):
        b, g = divmod(c, GROUPS)
        rows = slice(g * O, (g + 1) * O)
        in_maps.append({
            "xt": xts[b],
            "wqt": np.ascontiguousarray(Wq[rows, :].T).astype(bf16),
            "wkt": np.ascontiguousarray(Wk[rows, :].T).astype(bf16),
            "wvt": np.ascontiguousarray(Wv[rows, :].T).astype(bf16),
            "wot": np.ascontiguousarray(Wo[:, rows].T).astype(bf16),
        })
    return in_maps


def build_timed_callable(in_maps=None, loop_n=1):
    """Build the same sharded jit callable bass2jax uses, with inputs
    pre-placed on the 8 devices, for repeat-timing the NEFF execution.

    With loop_n=K the device program wraps the whole kernel body (including
    input DMAs) in a K-iteration hardware loop, so per-exec device time can
    be measured as a slope between two loop counts, cancelling the (large,
    noisy) axon dispatch overhead."""
    import jax
    import numpy as np
    from jax.sharding import Mesh, PartitionSpec, NamedSharding
    from jax.experimental.shard_map import shard_map
    from concourse import bass2jax, mybir

    nc = _get_program(loop_n)
    bass2jax.install_neuronx_cc_hook()

    if in_maps is None:
        import test as _t
        inputs, _ = _t.get_reference_data()
        in_maps = _make_in_maps(**inputs)

    partition_name = (
        nc.partition_id_tensor.name if nc.partition_id_tensor else None)
    in_names, out_names, out_avals, zero_shapes = [], [], [], []
    for alloc in nc.m.functions[0].allocations:
        if not isinstance(alloc, mybir.MemoryLocationSet):
            continue
        name = alloc.memorylocations[0].name
        if alloc.kind == "ExternalInput":
            if name != partition_name:
                in_names.append(name)
        elif alloc.kind == "ExternalOutput":
            out_names.append(name)
            shape = tuple(alloc.tensor_shape)
            out_avals.append(
                jax.core.ShapedArray(shape, mybir.dt.np(alloc.dtype)))
            zero_shapes.append((NCORES * shape[0], *shape[1:]))
    n_params = len(in_names)
    n_out = len(out_names)
    # operand order: inputs, donated zero outputs, partition ids (last, so
    # the hook's operand_ids[:-1] parameter-order check sees params 0..N-1)
    all_names = in_names + out_names
    if partition_name is not None:
        all_names = all_names + [partition_name]
    donate = tuple(range(n_params, n_params + n_out))

    def _body(*args):
        outs = bass2jax._bass_exec_p.bind(
            *args,
            out_avals=tuple(out_avals),
            in_names=tuple(all_names),
            out_names=tuple(out_names),
            lowering_input_output_aliases=(),
            sim_require_finite=True,
            sim_require_nnan=True,
            nc=nc,
        )
        return tuple(outs)

    devices = jax.devices()[:NCORES]
    mesh = Mesh(np.asarray(devices), ("core",))
    spec = PartitionSpec("core")
    n_extra = 1 if partition_name is not None else 0
    fn = jax.jit(
        shard_map(_body, mesh=mesh,
                  in_specs=(spec,) * (n_params + n_out + n_extra),
                  out_specs=(spec,) * n_out, check_rep=False),
        donate_argnums=donate, keep_unused=True,
    )
    sharding = NamedSharding(mesh, spec)
    concat_in = [
        jax.device_put(
            np.concatenate([np.asarray(in_maps[c][nm]) for c in range(NCORES)],
                           axis=0), sharding)
        for nm in in_names
    ]
    if partition_name is not None:
        pid = jax.device_put(
            np.arange(NCORES, dtype=np.uint32).reshape(NCORES, 1), sharding)
        fn_inner = fn
        fn = lambda *args: fn_inner(*args, pid)
    return fn, concat_in, [(s, sharding) for s in zero_shapes]
